# revision 13
# baseline (speedup 1.0000x reference)
"""AttentionLSTM Trainium2 kernel: data-parallel over batch on 8 NeuronCores.

Reference semantics (per batch element n):
  A_flat = A.reshape(N, H, 16); h0 = c0 = mean_p(A_flat)
  xWx = x @ Wx
  per step t:
    scores[p] = (h . A_flat[:, p]) / sqrt(H)
    w = softmax(scores); attn = A_flat @ w
    a = xWx_t + h @ Wh + attn @ Wattn + b
    i,f,o,g = sig/sig/sig/tanh of quarters; c = f*c + i*g; h = o*tanh(c)
  out[:, t, :] = h

Shapes: N=512, T=64, D=512, H=512 (4H=2048). 8 cores, 64 batch each.

Per core, 64 batch = two phase-staggered 32-element streams so one
stream's serial softmax/gate chains hide under the other's dense PE
work.

v2 design (vs baseline):
  - gm PSUM double-buffered per stream: step t+1's GEMM quads never
    wait on step t's gate ACT reads -> PE never idles a MID window ->
    HAM stays at K=8/8 instead of rethrottling every slot.
  - gate layout [128 x 512] with rows (hq*32+n), cols (gate,h128) via a
    host-side column permutation of W. Every gate/state elementwise op
    runs [128 x 128-384] instead of [32 x 512]: ~2-4x less V/S time.
  - softmax on the [128,4] q-blocked scores: one mask-mul + one grouped
    reduce (was 4+1), direct Exp ACT with accum_out giving the partial
    softmax denominator for free; cross-q denominator via a tiny
    replicating PE matmul (mQQ); normalization on ScalarE via per-
    partition scale.
  - single [128,128] CASTs for attnT and hT (were 4x each).
  - bf16 hN transposes (fp32 PE transpose is a 4-pass LOW_HIGH).
"""

import math
import sys

sys.path.insert(0, "/opt/trn_rl_repo")

import numpy as np
import ml_dtypes

import concourse.bass as bass
import concourse.mybir as mybir
from concourse.tile import TileContext
from concourse.bass_utils import run_bass_kernel_spmd

N, T, D, H = 512, 64, 512, 512
E = 4 * H  # 2048
NCORES = 8
NL = N // NCORES  # 64 batch per core
B = 32  # batch per stream
P16 = 16  # attention positions
NB = 4  # batch blocks of 8 per stream
SCALE = 1.0 / math.sqrt(H)

F32 = mybir.dt.float32
BF16 = mybir.dt.bfloat16


def build_nc(split_waits=True):
    nc = bass.Bass("TRN2", target_bir_lowering=False)

    # --- DRAM I/O ---
    xT_d = nc.declare_dram_parameter("xT", [T, D, NL], BF16, isOutput=False)
    AhT_d = nc.declare_dram_parameter("AhT", [H, 2, P16 * B], BF16, isOutput=False)
    APT_d = nc.declare_dram_parameter("APT", [128, 2, NB, H], BF16, isOutput=False)
    W2_d = nc.declare_dram_parameter("W2", [3 * H, E], BF16, isOutput=False)
    b2_d = nc.declare_dram_parameter("b2", [1, E], BF16, isOutput=False)
    c0_d = nc.declare_dram_parameter("c0", [2, 128, 128], F32, isOutput=False)
    h0T_d = nc.declare_dram_parameter("h0T", [H, NL], BF16, isOutput=False)
    i128_d = nc.declare_dram_parameter("i128", [128, 128], BF16, isOutput=False)
    mPN_d = nc.declare_dram_parameter("mPN", [128, 128], F32, isOutput=False)
    mBD_d = nc.declare_dram_parameter("mBD", [128, B], BF16, isOutput=False)
    d16x_d = nc.declare_dram_parameter("d16x", [128, 128], BF16, isOutput=False)
    mQQ_d = nc.declare_dram_parameter("mQQ", [128, 128], F32, isOutput=False)
    ones1_d = nc.declare_dram_parameter("ones1", [1, B], BF16, isOutput=False)
    out_d = nc.declare_dram_parameter("out", [NL, T, H], F32, isOutput=True)

    Sig = mybir.ActivationFunctionType.Sigmoid
    Tanh = mybir.ActivationFunctionType.Tanh

    with TileContext(nc) as tc:
        with (
            tc.tile_pool(name="wpool", bufs=1) as wpool,
            tc.tile_pool(name="state", bufs=1) as state,
            tc.tile_pool(name="xin", bufs=3) as xin,
            tc.tile_pool(name="work0", bufs=2) as work0,
            tc.tile_pool(name="work1", bufs=2) as work1,
            tc.tile_pool(name="psG", bufs=1, space="PSUM") as psG,
            tc.tile_pool(name="psM", bufs=1, space="PSUM") as psM,
        ):
            works = (work0, work1)
            # ---- persistent SBUF tensors (shared) ----
            W2_sb = wpool.tile([128, 12, E], BF16, tag="W2")
            nc.sync.dma_start(
                out=W2_sb[:], in_=W2_d.ap().rearrange("(k p) e -> p k e", p=128)
            )
            b2_sb = wpool.tile([1, E], BF16, tag="b2")
            nc.sync.dma_start(out=b2_sb[:], in_=b2_d[:])
            AhT_sb = wpool.tile([128, 4, 2, P16 * B], BF16, tag="AhT")
            nc.sync.dma_start(
                out=AhT_sb[:], in_=AhT_d.ap().rearrange("(k p) s f -> p k s f", p=128)
            )
            APT_sb = wpool.tile([128, 2, NB, H], BF16, tag="APT")
            nc.sync.dma_start(out=APT_sb[:], in_=APT_d[:])
            i128_sb = wpool.tile([128, 128], BF16, tag="i128")
            nc.sync.dma_start(out=i128_sb[:], in_=i128_d[:])
            mPN_sb = wpool.tile([128, 128], F32, tag="mPN")
            nc.sync.dma_start(out=mPN_sb[:], in_=mPN_d[:])
            mBD_sb = wpool.tile([128, B], BF16, tag="mBD")
            nc.sync.dma_start(out=mBD_sb[:], in_=mBD_d[:])
            d16x_sb = wpool.tile([128, 128], BF16, tag="d16x")
            nc.sync.dma_start(out=d16x_sb[:], in_=d16x_d[:])
            mQQ_sb = wpool.tile([128, 128], F32, tag="mQQ")
            nc.sync.dma_start(out=mQQ_sb[:], in_=mQQ_d[:])
            ones1_sb = wpool.tile([1, B], BF16, tag="ones1")
            nc.sync.dma_start(out=ones1_sb[:], in_=ones1_d[:])

            # ---- per-stream persistent state ----
            c_sb, hT_sb, wSn_sb, wSTs_sb, r128_sb, rd128_sb = (
                [None, None] for _ in range(6)
            )
            for s in range(2):
                c_sb[s] = state.tile([128, 128], F32, tag=f"c{s}", name=f"c{s}")
                nc.sync.dma_start(out=c_sb[s][:], in_=c0_d[s])
                hT_sb[s] = state.tile([128, 4, B], BF16, tag=f"hT{s}", name=f"hT{s}")
                nc.sync.dma_start(
                    out=hT_sb[s][:],
                    in_=h0T_d[:, s * B : (s + 1) * B].rearrange(
                        "(k p) n -> p k n", p=128
                    ),
                )
                wSn_sb[s] = state.tile([128, B], BF16, tag=f"wSn{s}", name=f"wSn{s}")
                nc.vector.memset(wSn_sb[s][:], 0.0)
                wSTs_sb[s] = state.tile([128, B], BF16, tag=f"wSTs{s}", name=f"wSTs{s}")
                r128_sb[s] = state.tile([128, 1], F32, tag=f"r128_{s}", name=f"r128_{s}")
                rd128_sb[s] = state.tile([128, 1], F32, tag=f"rd128_{s}", name=f"rd128_{s}")

            # ---- PSUM ----
            # gm double-buffered per stream: 4 banks.
            gm_ps = [
                [
                    psG.tile([128, 512], F32, tag=f"gm{s}{p}", name=f"gm{s}{p}")
                    for p in range(2)
                ]
                for s in range(2)
            ]
            # misc bank per stream (f32 words): X scores at [0:128),
            # at [128:256), wBD [256:288), den128 [288:289).
            ms_ps = [psM.tile([128, 512], F32, tag=f"ms{s}", name=f"ms{s}") for s in range(2)]
            # bf16 transpose target per stream
            tp_ps = [psM.tile([128, 128], BF16, tag=f"tp{s}", name=f"tp{s}") for s in range(2)]

            def gemm_quad(s, par, k, stat, start, stop):
                """One K-tile of the fused GEMM for all four hq row-groups.
                gm rows (hq*32+n), cols (gate,h128) -- W2 is column-permuted
                so member hq streams the contiguous hq-th 512-chunk."""
                for hq in range(4):
                    nc.tensor.matmul(
                        gm_ps[s][par][hq * B : (hq + 1) * B, :],
                        stat,
                        W2_sb[:, k, hq * 512 : (hq + 1) * 512],
                        start=start,
                        stop=stop,
                        skip_group_check=True,
                        tile_position=(0, hq * B),
                    )

            def bias_quad(s, par):
                for hq in range(4):
                    nc.tensor.matmul(
                        gm_ps[s][par][hq * B : (hq + 1) * B, :],
                        ones1_sb[:],
                        b2_sb[:, hq * 512 : (hq + 1) * 512],
                        start=True,
                        stop=False,
                        skip_group_check=True,
                        tile_position=(0, hq * B),
                    )

            def seg_a2x(s, t, xt):
                """Chain-free GEMM head: bias + xT K-tiles."""
                par = t % 2
                bias_quad(s, par)
                for k in range(4):
                    gemm_quad(s, par, k, xt[:, k, s * B : (s + 1) * B],
                              start=False, stop=False)

            def seg_b2(s, t, hNb):
                """hN transposes into hT (PE bf16) + one [128,128] copy."""
                if t >= T - 1:
                    return
                nc.tensor.transpose(tp_ps[s][:], hNb[:], i128_sb[:])
                nc.vector.tensor_copy(
                    hT_sb[s][:], tp_ps[s][:].rearrange("p (k n) -> p k n", n=B)
                )

            def seg_a1h(s, t):
                """scores + hT K-tiles, then mask/reduce/exp."""
                par = t % 2
                work = works[s]
                for j in range(4):
                    for q in range(4):
                        nc.tensor.matmul(
                            ms_ps[s][q * B : (q + 1) * B, 0:128],
                            hT_sb[s][:, j],
                            AhT_sb[:, j, s, q * 128 : (q + 1) * 128],
                            start=(j == 0),
                            stop=(j == 3),
                            skip_group_check=True,
                            tile_position=(0, q * B),
                        )
                for k in range(4):
                    gemm_quad(s, par, 4 + k, hT_sb[s][:, k], start=False, stop=False)

                Xm = work.tile([128, 128], F32, tag="Xm")
                nc.vector.tensor_mul(Xm[:], ms_ps[s][:, 0:128], mPN_sb[:])
                scSq = work.tile([128, 4], F32, tag="scSq")
                nc.vector.reduce_sum(
                    scSq[:],
                    Xm[:].rearrange("p (pl n) -> p pl n", n=B),
                    axis=mybir.AxisListType.X,
                )
                sgn = work.tile([128, 4], F32, tag="sgn")
                nc.scalar.activation(sgn[:], scSq[:], Sig, scale=-SCALE)
                rec = work.tile([128, 4], F32, tag="rec")
                nc.vector.reciprocal(rec[:], sgn[:])
                expS = work.tile([128, 4], F32, tag="expS")
                nc.vector.tensor_scalar(
                    expS[:], rec[:], -1.0, 0.0,
                    op0=mybir.AluOpType.add, op1=mybir.AluOpType.add,
                    accum_out=r128_sb[s][:],
                )
                return expS

            def seg_tail(s, t, expS):
                """den replication matmul + recip + normalize + transpose."""
                den = ms_ps[s][:, 288:289]
                nc.tensor.matmul(den, mQQ_sb[:], r128_sb[s][:], start=True, stop=True)
                nc.vector.reciprocal(rd128_sb[s][:], den)
                nc.vector.tensor_scalar_mul(wSn_sb[s][:, 0:4], expS[:], rd128_sb[s][:])
                nc.vector.transpose(wSTs_sb[s][:], wSn_sb[s][:])

            def seg_b1(s, t):
                """attention + attn GEMM K-tiles + gates/state update."""
                par = t % 2
                work = works[s]
                wBD_ps = ms_ps[s][:, 256:288]
                nc.tensor.matmul(
                    wBD_ps, d16x_sb[:], wSTs_sb[s][:], start=True, stop=True
                )
                wBDs = work.tile([128, B], BF16, tag="wBDs")
                nc.vector.tensor_mul(wBDs[:], wBD_ps, mBD_sb[:])

                for j in range(4):
                    at_j = ms_ps[s][:, 128 + j * B : 128 + (j + 1) * B]
                    for bb in range(NB):
                        nc.tensor.matmul(
                            at_j[:, bb * 8 : (bb + 1) * 8],
                            APT_sb[:, s, bb, j * 128 : (j + 1) * 128],
                            wBDs[:, bb * 8 : (bb + 1) * 8],
                            start=True,
                            stop=True,
                        )
                attnT = work.tile([128, 4, B], BF16, tag="attnT")
                nc.vector.tensor_copy(
                    attnT[:], ms_ps[s][:, 128:256].rearrange("p (k n) -> p k n", n=B)
                )

                for k in range(8, 12):
                    gemm_quad(s, par, k, attnT[:, k - 8], start=False, stop=(k == 11))

                gm = gm_ps[s][par]
                sg = work.tile([128, 384], BF16, tag="sg")
                nc.scalar.activation(sg[:], gm[:, 0:384], Sig)
                gg = work.tile([128, 128], BF16, tag="gg")
                nc.scalar.activation(gg[:], gm[:, 384:512], Tanh)
                fcp = work.tile([128, 128], F32, tag="fcp")
                nc.vector.tensor_mul(fcp[:], sg[:, 128:256], c_sb[s][:])
                igp = work.tile([128, 128], BF16, tag="igp")
                nc.vector.tensor_mul(igp[:], sg[:, 0:128], gg[:])
                nc.vector.tensor_add(c_sb[s][:], fcp[:], igp[:])
                tc_t = work.tile([128, 128], BF16, tag="tc")
                nc.scalar.activation(tc_t[:], c_sb[s][:], Tanh)
                hN = work.tile([128, 128], F32, tag="hN")
                nc.vector.tensor_mul(hN[:], sg[:, 256:384], tc_t[:])
                for hq in range(4):
                    nc.sync.dma_start(
                        out=out_d[s * B : (s + 1) * B, t, hq * 128 : (hq + 1) * 128],
                        in_=hN[hq * B : (hq + 1) * B, :],
                    )
                hNb = work.tile([128, 128], BF16, tag="hNb")
                nc.vector.tensor_copy(hNb[:], hN[:])
                return hNb

            # ---- slot pipeline over streams ----
            xts = {}
            xt0 = xin.tile([128, 4, NL], BF16, tag="xT")
            xts[0] = xt0
            nc.sync.dma_start(
                out=xt0[:], in_=xT_d[0].rearrange("(k p) n -> p k n", p=128)
            )

            exps, hnbs = {}, {}
            for u in range(2 * T + 2):
                sa = u % 2
                ta = u // 2
                sb = 1 - sa
                tb = (u - 1) // 2
                if ta < T:
                    if ta + 1 < T and (ta + 1) not in xts and sa == 1:
                        xtn = xin.tile([128, 4, NL], BF16, tag="xT")
                        xts[ta + 1] = xtn
                        nc.sync.dma_start(
                            out=xtn[:],
                            in_=xT_d[ta + 1].rearrange("(k p) n -> p k n", p=128),
                        )
                    seg_a2x(sa, ta, xts[ta])
                if ta - 1 >= 0 and (sa, ta - 1) in hnbs:
                    seg_b2(sa, ta - 1, hnbs.pop((sa, ta - 1)))
                if ta < T:
                    exps[(sa, ta)] = seg_a1h(sa, ta)
                if u >= 1 and tb < T:
                    hnbs[(sb, tb)] = seg_b1(sb, tb)
                if ta < T:
                    seg_tail(sa, ta, exps.pop((sa, ta)))

    if split_waits:
        _split_matmul_waits(nc)
    return nc


def _split_matmul_waits(nc):
    """Several TPB instruction encodings accept only one sync-wait command;
    hoist excess waits onto an inserted same-engine drain."""
    cnt = 0
    for f in nc.m.functions:
        for blk in f.blocks:
            new_insts = []
            for ins in blk.instructions:
                if (
                    ins.sync_info is not None
                    and ins.sync_info.on_wait
                    and len(ins.sync_info.on_wait) > 1
                ):
                    waits = list(ins.sync_info.on_wait)
                    for w in waits[:-1]:
                        cnt += 1
                        d = mybir.InstDrain(
                            name=f"I-mmw{cnt}", ins=[], outs=[],
                            engine=ins.engine,
                        )
                        d.sync_info = mybir.SyncInfo(on_wait=[w], on_update=[])
                        new_insts.append(d)
                    ins.sync_info = mybir.SyncInfo(
                        on_wait=[waits[-1]], on_update=list(ins.sync_info.on_update or [])
                    )
                new_insts.append(ins)
            blk.instructions = new_insts


def _prep_core_inputs(x_i, A_i, Wx, Wh, Wattn, b):
    """Host-side layout prep for one core's shard (x_i: (64,T,D), A_i: (64,H,4,4))."""
    nl = x_i.shape[0]
    A_flat = A_i.reshape(nl, H, P16)
    h0 = A_flat.mean(axis=2).astype(np.float32)  # (64, H)

    xT = np.ascontiguousarray(x_i.transpose(1, 2, 0)).astype(np.float32)  # (T, D, 64)
    # AhT[h, s, p*32+n] = A_flat[32s+n, h, p]
    AhT = np.ascontiguousarray(
        A_flat.transpose(1, 2, 0).reshape(H, P16, 2, B).transpose(0, 2, 1, 3)
        .reshape(H, 2, P16 * B)
    ).astype(np.float32)
    # APT[p*8+r, s, b, h] = A_flat[32s + 8b + r, h, p]
    APT = np.ascontiguousarray(
        A_flat.reshape(2, NB, 8, H, P16).transpose(4, 2, 0, 1, 3)
        .reshape(128, 2, NB, H)
    ).astype(np.float32)
    # W columns permuted: W2[:, hq*512 + g*128 + h1] = W[:, g*512 + hq*128 + h1]
    W = np.concatenate([Wx, Wh, Wattn], axis=0).astype(np.float32)  # (1536, E)
    W2 = np.ascontiguousarray(
        W.reshape(3 * H, 4, 4, 128).transpose(0, 2, 1, 3).reshape(3 * H, E)
    )
    b2 = np.ascontiguousarray(
        b.reshape(4, 4, 128).transpose(1, 0, 2).reshape(1, E)
    ).astype(np.float32)
    # c0[s, hq*32+n, h1] = h0[s*32+n, hq*128+h1]
    c0 = np.ascontiguousarray(
        h0.reshape(2, B, 4, 128).transpose(0, 2, 1, 3).reshape(2, 128, 128)
    )
    i128 = np.eye(128, dtype=np.float32)
    # mPN[q*32+m, pl*32+n] = (n == m)
    mPN = np.tile(np.tile(np.eye(B, dtype=np.float32), (1, 4)), (4, 1))  # (128, 128)
    mBD = np.tile(np.tile(np.eye(8, dtype=np.float32), (1, NB)), (P16, 1))  # (128,32)
    # d16x[q*32+pl, p*8+r] = (p == q*4+pl), pl<4
    d16x = np.zeros((128, 128), dtype=np.float32)
    for p in range(P16):
        q, pl = p // 4, p % 4
        d16x[q * 32 + pl, p * 8 : (p + 1) * 8] = 1.0
    mQQ = np.tile(np.eye(B, dtype=np.float32), (4, 4))  # (128, 128)
    ones1 = np.ones((1, B), dtype=np.float32)
    bf16 = ml_dtypes.bfloat16
    return {
        "xT": xT.astype(bf16),
        "AhT": AhT.astype(bf16),
        "APT": APT.astype(bf16),
        "W2": W2.astype(bf16),
        "b2": b2.astype(bf16),
        "c0": c0,
        "h0T": np.ascontiguousarray(h0.T).astype(bf16),
        "i128": i128.astype(bf16),
        "mPN": mPN,
        "mBD": mBD.astype(bf16),
        "d16x": d16x.astype(bf16),
        "mQQ": mQQ,
        "ones1": ones1.astype(bf16),
    }


_NC_CACHE = {}


def kernel(x, A, Wx, Wh, Wattn, b, _trace=False):
    x = np.asarray(x, dtype=np.float32)
    A = np.asarray(A, dtype=np.float32)
    Wx = np.asarray(Wx, dtype=np.float32)
    Wh = np.asarray(Wh, dtype=np.float32)
    Wattn = np.asarray(Wattn, dtype=np.float32)
    b = np.asarray(b, dtype=np.float32)

    if "nc" not in _NC_CACHE:
        _NC_CACHE["nc"] = build_nc()
    nc = _NC_CACHE["nc"]

    in_maps = []
    for i in range(NCORES):
        sl = slice(i * NL, (i + 1) * NL)
        in_maps.append(_prep_core_inputs(x[sl], A[sl], Wx, Wh, Wattn, b))

    res = run_bass_kernel_spmd(
        nc, in_maps, core_ids=list(range(NCORES)), trace=_trace
    )
    outs = [res.results[i]["out"] for i in range(NCORES)]
    full = np.concatenate(outs, axis=0)  # (N, T, H)
    if _trace:
        kernel.last_exec_time_ns = res.exec_time_ns
        kernel.last_profile = res.profile_json
    return full


kernel.last_exec_time_ns = None
kernel.last_profile = None


# revision 14
# speedup vs baseline: 1.2395x; 1.2395x over previous
"""AttentionLSTM Trainium2 kernel: data-parallel over batch on 8 NeuronCores.

Reference semantics (per batch element n):
  A_flat = A.reshape(N, H, 16); h0 = c0 = mean_p(A_flat)
  xWx = x @ Wx
  per step t:
    scores[p] = (h . A_flat[:, p]) / sqrt(H)
    w = softmax(scores); attn = A_flat @ w
    a = xWx_t + h @ Wh + attn @ Wattn + b
    i,f,o,g = sig/sig/sig/tanh of quarters; c = f*c + i*g; h = o*tanh(c)
  out[:, t, :] = h

Shapes: N=512, T=64, D=512, H=512 (4H=2048). 8 cores, 64 batch each.

Per core, 64 batch = two phase-staggered 32-element streams so one
stream's serial softmax/gate chains hide under the other's dense PE
work.

v2 design (vs baseline):
  - gm PSUM double-buffered per stream: step t+1's GEMM quads never
    wait on step t's gate ACT reads -> PE never idles a MID window ->
    HAM stays at K=8/8 instead of rethrottling every slot.
  - gate layout [128 x 512] with rows (hq*32+n), cols (gate,h128) via a
    host-side column permutation of W. Every gate/state elementwise op
    runs [128 x 128-384] instead of [32 x 512]: ~2-4x less V/S time.
  - softmax on the [128,4] q-blocked scores: one mask-mul + one grouped
    reduce (was 4+1), direct Exp ACT with accum_out giving the partial
    softmax denominator for free; cross-q denominator via a tiny
    replicating PE matmul (mQQ); normalization on ScalarE via per-
    partition scale.
  - single [128,128] CASTs for attnT and hT (were 4x each).
  - bf16 hN transposes (fp32 PE transpose is a 4-pass LOW_HIGH).
"""

import math
import sys

sys.path.insert(0, "/opt/trn_rl_repo")

import numpy as np
import ml_dtypes

import concourse.bass as bass
import concourse.mybir as mybir
from concourse.tile import TileContext
from concourse.bass_utils import run_bass_kernel_spmd

N, T, D, H = 512, 64, 512, 512
E = 4 * H  # 2048
NCORES = 8
NL = N // NCORES  # 64 batch per core
B = 32  # batch per stream
P16 = 16  # attention positions
NB = 4  # batch blocks of 8 per stream
SCALE = 1.0 / math.sqrt(H)

F32 = mybir.dt.float32
BF16 = mybir.dt.bfloat16


def build_nc(split_waits=True):
    nc = bass.Bass("TRN2", target_bir_lowering=False)

    # --- DRAM I/O ---
    xT_d = nc.declare_dram_parameter("xT", [T, D, NL], BF16, isOutput=False)
    AhT_d = nc.declare_dram_parameter("AhT", [H, 2, P16 * B], BF16, isOutput=False)
    APT_d = nc.declare_dram_parameter("APT", [128, 2, NB, H], BF16, isOutput=False)
    W2_d = nc.declare_dram_parameter("W2", [3 * H, E], BF16, isOutput=False)
    b2_d = nc.declare_dram_parameter("b2", [1, E], BF16, isOutput=False)
    c0_d = nc.declare_dram_parameter("c0", [2, 128, 128], F32, isOutput=False)
    h0T_d = nc.declare_dram_parameter("h0T", [H, NL], BF16, isOutput=False)
    i128_d = nc.declare_dram_parameter("i128", [128, 128], BF16, isOutput=False)
    mPN_d = nc.declare_dram_parameter("mPN", [128, 128], F32, isOutput=False)
    mBD_d = nc.declare_dram_parameter("mBD", [128, B], BF16, isOutput=False)
    d16x_d = nc.declare_dram_parameter("d16x", [128, 128], BF16, isOutput=False)
    mQQ_d = nc.declare_dram_parameter("mQQ", [128, 128], F32, isOutput=False)
    ones1_d = nc.declare_dram_parameter("ones1", [1, B], BF16, isOutput=False)
    out_d = nc.declare_dram_parameter("out", [NL, T, H], F32, isOutput=True)

    Sig = mybir.ActivationFunctionType.Sigmoid
    Tanh = mybir.ActivationFunctionType.Tanh

    with TileContext(nc) as tc:
        with (
            tc.tile_pool(name="wpool", bufs=1) as wpool,
            tc.tile_pool(name="state", bufs=1) as state,
            tc.tile_pool(name="xin", bufs=3) as xin,
            tc.tile_pool(name="work0", bufs=2) as work0,
            tc.tile_pool(name="work1", bufs=2) as work1,
            tc.tile_pool(name="psG", bufs=1, space="PSUM") as psG,
            tc.tile_pool(name="psM", bufs=1, space="PSUM") as psM,
        ):
            works = (work0, work1)
            # ---- persistent SBUF tensors (shared) ----
            W2_sb = wpool.tile([128, 12, E], BF16, tag="W2")
            nc.sync.dma_start(
                out=W2_sb[:], in_=W2_d.ap().rearrange("(k p) e -> p k e", p=128)
            )
            b2_sb = wpool.tile([1, E], BF16, tag="b2")
            nc.sync.dma_start(out=b2_sb[:], in_=b2_d[:])
            AhT_sb = wpool.tile([128, 4, 2, P16 * B], BF16, tag="AhT")
            nc.sync.dma_start(
                out=AhT_sb[:], in_=AhT_d.ap().rearrange("(k p) s f -> p k s f", p=128)
            )
            APT_sb = wpool.tile([128, 2, NB, H], BF16, tag="APT")
            nc.sync.dma_start(out=APT_sb[:], in_=APT_d[:])
            i128_sb = wpool.tile([128, 128], BF16, tag="i128")
            nc.sync.dma_start(out=i128_sb[:], in_=i128_d[:])
            mPN_sb = wpool.tile([128, 128], F32, tag="mPN")
            nc.sync.dma_start(out=mPN_sb[:], in_=mPN_d[:])
            mBD_sb = wpool.tile([128, B], BF16, tag="mBD")
            nc.sync.dma_start(out=mBD_sb[:], in_=mBD_d[:])
            d16x_sb = wpool.tile([128, 128], BF16, tag="d16x")
            nc.sync.dma_start(out=d16x_sb[:], in_=d16x_d[:])
            mQQ_sb = wpool.tile([128, 128], F32, tag="mQQ")
            nc.sync.dma_start(out=mQQ_sb[:], in_=mQQ_d[:])
            ones1_sb = wpool.tile([1, B], BF16, tag="ones1")
            nc.sync.dma_start(out=ones1_sb[:], in_=ones1_d[:])

            # ---- per-stream persistent state ----
            c_sb, hT_sb, wSn_sb, wSTs_sb, r128_sb, rd128_sb = (
                [None, None] for _ in range(6)
            )
            for s in range(2):
                c_sb[s] = state.tile([128, 128], F32, tag=f"c{s}", name=f"c{s}")
                nc.sync.dma_start(out=c_sb[s][:], in_=c0_d[s])
                hT_sb[s] = state.tile([128, 4, B], BF16, tag=f"hT{s}", name=f"hT{s}")
                nc.sync.dma_start(
                    out=hT_sb[s][:],
                    in_=h0T_d[:, s * B : (s + 1) * B].rearrange(
                        "(k p) n -> p k n", p=128
                    ),
                )
                wSn_sb[s] = state.tile([128, B], BF16, tag=f"wSn{s}", name=f"wSn{s}")
                nc.vector.memset(wSn_sb[s][:], 0.0)
                wSTs_sb[s] = state.tile([128, B], BF16, tag=f"wSTs{s}", name=f"wSTs{s}")
                r128_sb[s] = state.tile([128, 1], F32, tag=f"r128_{s}", name=f"r128_{s}")
                rd128_sb[s] = state.tile([128, 1], F32, tag=f"rd128_{s}", name=f"rd128_{s}")

            # ---- PSUM ----
            # gm double-buffered per stream: 4 banks.
            gm_ps = [
                [
                    psG.tile([128, 512], F32, tag=f"gm{s}{p}", name=f"gm{s}{p}")
                    for p in range(2)
                ]
                for s in range(2)
            ]
            # misc bank per stream (f32 words): X scores at [0:128),
            # at [128:256), wBD [256:288), den128 [288:289).
            ms_ps = [psM.tile([128, 512], F32, tag=f"ms{s}", name=f"ms{s}") for s in range(2)]
            # bf16 transpose target per stream
            tp_ps = [psM.tile([128, 128], BF16, tag=f"tp{s}", name=f"tp{s}") for s in range(2)]

            def gemm_quad(s, par, k, stat, start, stop):
                """One K-tile of the fused GEMM for all four hq row-groups.
                gm rows (hq*32+n), cols (gate,h128) -- W2 is column-permuted
                so member hq streams the contiguous hq-th 512-chunk."""
                for hq in range(4):
                    nc.tensor.matmul(
                        gm_ps[s][par][hq * B : (hq + 1) * B, :],
                        stat,
                        W2_sb[:, k, hq * 512 : (hq + 1) * 512],
                        start=start,
                        stop=stop,
                        skip_group_check=True,
                        tile_position=(0, hq * B),
                    )

            def bias_quad(s, par):
                for hq in range(4):
                    nc.tensor.matmul(
                        gm_ps[s][par][hq * B : (hq + 1) * B, :],
                        ones1_sb[:],
                        b2_sb[:, hq * 512 : (hq + 1) * 512],
                        start=True,
                        stop=False,
                        skip_group_check=True,
                        tile_position=(0, hq * B),
                    )

            def seg_a2x(s, t, xt):
                """Chain-free GEMM head: bias + xT K-tiles."""
                par = t % 2
                bias_quad(s, par)
                for k in range(4):
                    gemm_quad(s, par, k, xt[:, k, s * B : (s + 1) * B],
                              start=False, stop=False)

            def seg_b2(s, t, hNb):
                """hN transposes into hT (PE bf16) + one [128,128] copy."""
                if t >= T - 1:
                    return
                nc.tensor.transpose(tp_ps[s][:], hNb[:], i128_sb[:])
                nc.vector.tensor_copy(
                    hT_sb[s][:], tp_ps[s][:].rearrange("p (k n) -> p k n", n=B)
                )

            def seg_a1h(s, t):
                """scores + hT K-tiles, then mask/reduce/exp."""
                par = t % 2
                work = works[s]
                for j in range(4):
                    for q in range(4):
                        nc.tensor.matmul(
                            ms_ps[s][q * B : (q + 1) * B, 0:128],
                            hT_sb[s][:, j],
                            AhT_sb[:, j, s, q * 128 : (q + 1) * 128],
                            start=(j == 0),
                            stop=(j == 3),
                            skip_group_check=True,
                            tile_position=(0, q * B),
                        )
                for k in range(4):
                    gemm_quad(s, par, 4 + k, hT_sb[s][:, k], start=False, stop=False)

                Xm = work.tile([128, 128], F32, tag="Xm")
                nc.vector.tensor_mul(Xm[:], ms_ps[s][:, 0:128], mPN_sb[:])
                scSq = work.tile([128, 4], F32, tag="scSq")
                nc.vector.reduce_sum(
                    scSq[:],
                    Xm[:].rearrange("p (pl n) -> p pl n", n=B),
                    axis=mybir.AxisListType.X,
                )
                sgn = work.tile([128, 4], F32, tag="sgn")
                nc.scalar.activation(sgn[:], scSq[:], Sig, scale=-SCALE)
                rec = work.tile([128, 4], F32, tag="rec")
                nc.vector.reciprocal(rec[:], sgn[:])
                expS = work.tile([128, 4], F32, tag="expS")
                nc.vector.tensor_scalar(
                    expS[:], rec[:], -1.0, 0.0,
                    op0=mybir.AluOpType.add, op1=mybir.AluOpType.add,
                    accum_out=r128_sb[s][:],
                )
                return expS

            def seg_tail(s, t, expS):
                """den replication matmul + recip + normalize + transpose."""
                den = ms_ps[s][:, 288:289]
                nc.tensor.matmul(den, mQQ_sb[:], r128_sb[s][:], start=True, stop=True)
                nc.vector.reciprocal(rd128_sb[s][:], den)
                nc.vector.tensor_scalar_mul(wSn_sb[s][:, 0:4], expS[:], rd128_sb[s][:])
                nc.vector.transpose(wSTs_sb[s][:], wSn_sb[s][:])

            def seg_b1(s, t):
                """attention + attn GEMM K-tiles + gates/state update."""
                par = t % 2
                work = works[s]
                wBD_ps = ms_ps[s][:, 256:288]
                nc.tensor.matmul(
                    wBD_ps, d16x_sb[:], wSTs_sb[s][:], start=True, stop=True
                )
                wBDs = work.tile([128, B], BF16, tag="wBDs")
                nc.vector.tensor_mul(wBDs[:], wBD_ps, mBD_sb[:])

                for j in range(4):
                    at_j = ms_ps[s][:, 128 + j * B : 128 + (j + 1) * B]
                    for bb in range(NB):
                        nc.tensor.matmul(
                            at_j[:, bb * 8 : (bb + 1) * 8],
                            APT_sb[:, s, bb, j * 128 : (j + 1) * 128],
                            wBDs[:, bb * 8 : (bb + 1) * 8],
                            start=True,
                            stop=True,
                        )
                attnT = work.tile([128, 4, B], BF16, tag="attnT")
                nc.vector.tensor_copy(
                    attnT[:], ms_ps[s][:, 128:256].rearrange("p (k n) -> p k n", n=B)
                )

                for k in range(8, 12):
                    gemm_quad(s, par, k, attnT[:, k - 8], start=False, stop=(k == 11))

                gm = gm_ps[s][par]
                sg = work.tile([128, 384], BF16, tag="sg")
                nc.scalar.activation(sg[:], gm[:, 0:384], Sig)
                gg = work.tile([128, 128], BF16, tag="gg")
                nc.scalar.activation(gg[:], gm[:, 384:512], Tanh)
                fcp = work.tile([128, 128], F32, tag="fcp")
                nc.vector.tensor_mul(fcp[:], sg[:, 128:256], c_sb[s][:])
                igp = work.tile([128, 128], BF16, tag="igp")
                nc.vector.tensor_mul(igp[:], sg[:, 0:128], gg[:])
                nc.vector.tensor_add(c_sb[s][:], fcp[:], igp[:])
                tc_t = work.tile([128, 128], BF16, tag="tc")
                nc.scalar.activation(tc_t[:], c_sb[s][:], Tanh)
                hN = work.tile([128, 128], F32, tag="hN")
                nc.vector.tensor_mul(hN[:], sg[:, 256:384], tc_t[:])
                for hq in range(4):
                    nc.sync.dma_start(
                        out=out_d[s * B : (s + 1) * B, t, hq * 128 : (hq + 1) * 128],
                        in_=hN[hq * B : (hq + 1) * B, :],
                    )
                hNb = work.tile([128, 128], BF16, tag="hNb")
                nc.vector.tensor_copy(hNb[:], hN[:])
                return hNb

            # ---- slot pipeline over streams ----
            xts = {}
            xt0 = xin.tile([128, 4, NL], BF16, tag="xT")
            xts[0] = xt0
            nc.sync.dma_start(
                out=xt0[:], in_=xT_d[0].rearrange("(k p) n -> p k n", p=128)
            )

            exps, hnbs = {}, {}
            for u in range(2 * T + 2):
                sa = u % 2
                ta = u // 2
                sb = 1 - sa
                tb = (u - 1) // 2
                if ta < T:
                    if ta + 1 < T and (ta + 1) not in xts and sa == 1:
                        xtn = xin.tile([128, 4, NL], BF16, tag="xT")
                        xts[ta + 1] = xtn
                        nc.sync.dma_start(
                            out=xtn[:],
                            in_=xT_d[ta + 1].rearrange("(k p) n -> p k n", p=128),
                        )
                    seg_a2x(sa, ta, xts[ta])
                if u >= 1 and tb < T:
                    hnbs[(sb, tb)] = seg_b1(sb, tb)
                if ta - 1 >= 0 and (sa, ta - 1) in hnbs:
                    seg_b2(sa, ta - 1, hnbs.pop((sa, ta - 1)))
                if ta < T:
                    exps[(sa, ta)] = seg_a1h(sa, ta)
                    seg_tail(sa, ta, exps.pop((sa, ta)))

    if split_waits:
        _split_matmul_waits(nc)
    return nc


def _split_matmul_waits(nc):
    """Several TPB instruction encodings accept only one sync-wait command;
    hoist excess waits onto an inserted same-engine drain."""
    cnt = 0
    for f in nc.m.functions:
        for blk in f.blocks:
            new_insts = []
            for ins in blk.instructions:
                if (
                    ins.sync_info is not None
                    and ins.sync_info.on_wait
                    and len(ins.sync_info.on_wait) > 1
                ):
                    waits = list(ins.sync_info.on_wait)
                    for w in waits[:-1]:
                        cnt += 1
                        d = mybir.InstDrain(
                            name=f"I-mmw{cnt}", ins=[], outs=[],
                            engine=ins.engine,
                        )
                        d.sync_info = mybir.SyncInfo(on_wait=[w], on_update=[])
                        new_insts.append(d)
                    ins.sync_info = mybir.SyncInfo(
                        on_wait=[waits[-1]], on_update=list(ins.sync_info.on_update or [])
                    )
                new_insts.append(ins)
            blk.instructions = new_insts


def _prep_core_inputs(x_i, A_i, Wx, Wh, Wattn, b):
    """Host-side layout prep for one core's shard (x_i: (64,T,D), A_i: (64,H,4,4))."""
    nl = x_i.shape[0]
    A_flat = A_i.reshape(nl, H, P16)
    h0 = A_flat.mean(axis=2).astype(np.float32)  # (64, H)

    xT = np.ascontiguousarray(x_i.transpose(1, 2, 0)).astype(np.float32)  # (T, D, 64)
    # AhT[h, s, p*32+n] = A_flat[32s+n, h, p]
    AhT = np.ascontiguousarray(
        A_flat.transpose(1, 2, 0).reshape(H, P16, 2, B).transpose(0, 2, 1, 3)
        .reshape(H, 2, P16 * B)
    ).astype(np.float32)
    # APT[p*8+r, s, b, h] = A_flat[32s + 8b + r, h, p]
    APT = np.ascontiguousarray(
        A_flat.reshape(2, NB, 8, H, P16).transpose(4, 2, 0, 1, 3)
        .reshape(128, 2, NB, H)
    ).astype(np.float32)
    # W columns permuted: W2[:, hq*512 + g*128 + h1] = W[:, g*512 + hq*128 + h1]
    W = np.concatenate([Wx, Wh, Wattn], axis=0).astype(np.float32)  # (1536, E)
    W2 = np.ascontiguousarray(
        W.reshape(3 * H, 4, 4, 128).transpose(0, 2, 1, 3).reshape(3 * H, E)
    )
    b2 = np.ascontiguousarray(
        b.reshape(4, 4, 128).transpose(1, 0, 2).reshape(1, E)
    ).astype(np.float32)
    # c0[s, hq*32+n, h1] = h0[s*32+n, hq*128+h1]
    c0 = np.ascontiguousarray(
        h0.reshape(2, B, 4, 128).transpose(0, 2, 1, 3).reshape(2, 128, 128)
    )
    i128 = np.eye(128, dtype=np.float32)
    # mPN[q*32+m, pl*32+n] = (n == m)
    mPN = np.tile(np.tile(np.eye(B, dtype=np.float32), (1, 4)), (4, 1))  # (128, 128)
    mBD = np.tile(np.tile(np.eye(8, dtype=np.float32), (1, NB)), (P16, 1))  # (128,32)
    # d16x[q*32+pl, p*8+r] = (p == q*4+pl), pl<4
    d16x = np.zeros((128, 128), dtype=np.float32)
    for p in range(P16):
        q, pl = p // 4, p % 4
        d16x[q * 32 + pl, p * 8 : (p + 1) * 8] = 1.0
    mQQ = np.tile(np.eye(B, dtype=np.float32), (4, 4))  # (128, 128)
    ones1 = np.ones((1, B), dtype=np.float32)
    bf16 = ml_dtypes.bfloat16
    return {
        "xT": xT.astype(bf16),
        "AhT": AhT.astype(bf16),
        "APT": APT.astype(bf16),
        "W2": W2.astype(bf16),
        "b2": b2.astype(bf16),
        "c0": c0,
        "h0T": np.ascontiguousarray(h0.T).astype(bf16),
        "i128": i128.astype(bf16),
        "mPN": mPN,
        "mBD": mBD.astype(bf16),
        "d16x": d16x.astype(bf16),
        "mQQ": mQQ,
        "ones1": ones1.astype(bf16),
    }


_NC_CACHE = {}


def kernel(x, A, Wx, Wh, Wattn, b, _trace=False):
    x = np.asarray(x, dtype=np.float32)
    A = np.asarray(A, dtype=np.float32)
    Wx = np.asarray(Wx, dtype=np.float32)
    Wh = np.asarray(Wh, dtype=np.float32)
    Wattn = np.asarray(Wattn, dtype=np.float32)
    b = np.asarray(b, dtype=np.float32)

    if "nc" not in _NC_CACHE:
        _NC_CACHE["nc"] = build_nc()
    nc = _NC_CACHE["nc"]

    in_maps = []
    for i in range(NCORES):
        sl = slice(i * NL, (i + 1) * NL)
        in_maps.append(_prep_core_inputs(x[sl], A[sl], Wx, Wh, Wattn, b))

    res = run_bass_kernel_spmd(
        nc, in_maps, core_ids=list(range(NCORES)), trace=_trace
    )
    outs = [res.results[i]["out"] for i in range(NCORES)]
    full = np.concatenate(outs, axis=0)  # (N, T, H)
    if _trace:
        kernel.last_exec_time_ns = res.exec_time_ns
        kernel.last_profile = res.profile_json
    return full


kernel.last_exec_time_ns = None
kernel.last_profile = None


# revision 15
# speedup vs baseline: 1.3769x; 1.1108x over previous
"""AttentionLSTM Trainium2 kernel: data-parallel over batch on 8 NeuronCores.

Reference semantics (per batch element n):
  A_flat = A.reshape(N, H, 16); h0 = c0 = mean_p(A_flat)
  xWx = x @ Wx
  per step t:
    scores[p] = (h . A_flat[:, p]) / sqrt(H)
    w = softmax(scores); attn = A_flat @ w
    a = xWx_t + h @ Wh + attn @ Wattn + b
    i,f,o,g = sig/sig/sig/tanh of quarters; c = f*c + i*g; h = o*tanh(c)
  out[:, t, :] = h

Shapes: N=512, T=64, D=512, H=512 (4H=2048). 8 cores, 64 batch each.

Per core, 64 batch = two phase-staggered 32-element streams so one
stream's serial softmax/gate chains hide under the other's dense PE
work.

v2 design (vs baseline):
  - gm PSUM double-buffered per stream: step t+1's GEMM quads never
    wait on step t's gate ACT reads -> PE never idles a MID window ->
    HAM stays at K=8/8 instead of rethrottling every slot.
  - gate layout [128 x 512] with rows (hq*32+n), cols (gate,h128) via a
    host-side column permutation of W. Every gate/state elementwise op
    runs [128 x 128-384] instead of [32 x 512]: ~2-4x less V/S time.
  - softmax on the [128,4] q-blocked scores: one mask-mul + one grouped
    reduce (was 4+1), direct Exp ACT with accum_out giving the partial
    softmax denominator for free; cross-q denominator via a tiny
    replicating PE matmul (mQQ); normalization on ScalarE via per-
    partition scale.
  - single [128,128] CASTs for attnT and hT (were 4x each).
  - bf16 hN transposes (fp32 PE transpose is a 4-pass LOW_HIGH).
"""

import math
import sys

sys.path.insert(0, "/opt/trn_rl_repo")

import numpy as np
import ml_dtypes

import concourse.bass as bass
import concourse.mybir as mybir
from concourse.tile import TileContext
from concourse.bass_utils import run_bass_kernel_spmd

N, T, D, H = 512, 64, 512, 512
E = 4 * H  # 2048
NCORES = 8
NL = N // NCORES  # 64 batch per core
B = 32  # batch per stream
P16 = 16  # attention positions
NB = 4  # batch blocks of 8 per stream
SCALE = 1.0 / math.sqrt(H)

F32 = mybir.dt.float32
BF16 = mybir.dt.bfloat16


def build_nc(split_waits=True):
    nc = bass.Bass("TRN2", target_bir_lowering=False)

    # --- DRAM I/O ---
    xT_d = nc.declare_dram_parameter("xT", [T, D, NL], BF16, isOutput=False)
    AhT_d = nc.declare_dram_parameter("AhT", [H, 2, P16 * B], BF16, isOutput=False)
    APT_d = nc.declare_dram_parameter("APT", [128, 2, NB, H], BF16, isOutput=False)
    W2_d = nc.declare_dram_parameter("W2", [3 * H, E], BF16, isOutput=False)
    b2_d = nc.declare_dram_parameter("b2", [1, E], BF16, isOutput=False)
    c0_d = nc.declare_dram_parameter("c0", [2, 128, 128], F32, isOutput=False)
    h0T_d = nc.declare_dram_parameter("h0T", [H, NL], BF16, isOutput=False)
    i128_d = nc.declare_dram_parameter("i128", [128, 128], BF16, isOutput=False)
    mPN_d = nc.declare_dram_parameter("mPN", [128, 128], F32, isOutput=False)
    mBD_d = nc.declare_dram_parameter("mBD", [128, B], BF16, isOutput=False)
    d16x_d = nc.declare_dram_parameter("d16x", [128, 128], BF16, isOutput=False)
    mQQ_d = nc.declare_dram_parameter("mQQ", [128, 128], F32, isOutput=False)
    ones1_d = nc.declare_dram_parameter("ones1", [1, B], BF16, isOutput=False)
    out_d = nc.declare_dram_parameter("out", [NL, T, H], F32, isOutput=True)

    Sig = mybir.ActivationFunctionType.Sigmoid
    Tanh = mybir.ActivationFunctionType.Tanh

    with TileContext(nc) as tc:
        with (
            tc.tile_pool(name="wpool", bufs=1) as wpool,
            tc.tile_pool(name="state", bufs=1) as state,
            tc.tile_pool(name="xin", bufs=3) as xin,
            tc.tile_pool(name="work0", bufs=2) as work0,
            tc.tile_pool(name="work1", bufs=2) as work1,
            tc.tile_pool(name="psG", bufs=1, space="PSUM") as psG,
            tc.tile_pool(name="psM", bufs=1, space="PSUM") as psM,
        ):
            works = (work0, work1)
            # ---- persistent SBUF tensors (shared) ----
            W2_sb = wpool.tile([128, 12, E], BF16, tag="W2")
            nc.sync.dma_start(
                out=W2_sb[:], in_=W2_d.ap().rearrange("(k p) e -> p k e", p=128)
            )
            b2_sb = wpool.tile([1, E], BF16, tag="b2")
            nc.sync.dma_start(out=b2_sb[:], in_=b2_d[:])
            AhT_sb = wpool.tile([128, 4, 2, P16 * B], BF16, tag="AhT")
            nc.sync.dma_start(
                out=AhT_sb[:], in_=AhT_d.ap().rearrange("(k p) s f -> p k s f", p=128)
            )
            APT_sb = wpool.tile([128, 2, NB, H], BF16, tag="APT")
            nc.sync.dma_start(out=APT_sb[:], in_=APT_d[:])
            i128_sb = wpool.tile([128, 128], BF16, tag="i128")
            nc.sync.dma_start(out=i128_sb[:], in_=i128_d[:])
            mPN_sb = wpool.tile([128, 128], F32, tag="mPN")
            nc.sync.dma_start(out=mPN_sb[:], in_=mPN_d[:])
            mBD_sb = wpool.tile([128, B], BF16, tag="mBD")
            nc.sync.dma_start(out=mBD_sb[:], in_=mBD_d[:])
            d16x_sb = wpool.tile([128, 128], BF16, tag="d16x")
            nc.sync.dma_start(out=d16x_sb[:], in_=d16x_d[:])
            mQQ_sb = wpool.tile([128, 128], F32, tag="mQQ")
            nc.sync.dma_start(out=mQQ_sb[:], in_=mQQ_d[:])
            ones1_sb = wpool.tile([1, B], BF16, tag="ones1")
            nc.sync.dma_start(out=ones1_sb[:], in_=ones1_d[:])

            # ---- per-stream persistent state ----
            c_sb, hT_sb, wSn_sb, wSTs_sb, r128_sb, rd128_sb = (
                [None, None] for _ in range(6)
            )
            for s in range(2):
                c_sb[s] = state.tile([128, 128], F32, tag=f"c{s}", name=f"c{s}")
                nc.sync.dma_start(out=c_sb[s][:], in_=c0_d[s])
                hT_sb[s] = state.tile([128, 4, B], BF16, tag=f"hT{s}", name=f"hT{s}")
                nc.sync.dma_start(
                    out=hT_sb[s][:],
                    in_=h0T_d[:, s * B : (s + 1) * B].rearrange(
                        "(k p) n -> p k n", p=128
                    ),
                )
                wSn_sb[s] = state.tile([128, B], BF16, tag=f"wSn{s}", name=f"wSn{s}")
                nc.vector.memset(wSn_sb[s][:], 0.0)
                wSTs_sb[s] = state.tile([128, B], BF16, tag=f"wSTs{s}", name=f"wSTs{s}")
                r128_sb[s] = state.tile([128, 1], F32, tag=f"r128_{s}", name=f"r128_{s}")
                rd128_sb[s] = state.tile([128, 1], F32, tag=f"rd128_{s}", name=f"rd128_{s}")

            # ---- PSUM ----
            # gm double-buffered per stream: 4 banks.
            gm_ps = [
                [
                    psG.tile([128, 512], F32, tag=f"gm{s}{p}", name=f"gm{s}{p}")
                    for p in range(2)
                ]
                for s in range(2)
            ]
            # misc bank per stream (f32 words): X scores at [0:128),
            # at [128:256), wBD [256:288), den128 [288:289).
            ms_ps = [psM.tile([128, 512], F32, tag=f"ms{s}", name=f"ms{s}") for s in range(2)]
            # bf16 transpose target per stream
            tp_ps = [psM.tile([128, 128], BF16, tag=f"tp{s}", name=f"tp{s}") for s in range(2)]

            def gemm_quad(s, par, k, stat, start, stop):
                """One K-tile of the fused GEMM for all four hq row-groups.
                gm rows (hq*32+n), cols (gate,h128) -- W2 is column-permuted
                so member hq streams the contiguous hq-th 512-chunk."""
                for hq in range(4):
                    nc.tensor.matmul(
                        gm_ps[s][par][hq * B : (hq + 1) * B, :],
                        stat,
                        W2_sb[:, k, hq * 512 : (hq + 1) * 512],
                        start=start,
                        stop=stop,
                        skip_group_check=True,
                        tile_position=(0, hq * B),
                    )

            def bias_quad(s, par):
                for hq in range(4):
                    nc.tensor.matmul(
                        gm_ps[s][par][hq * B : (hq + 1) * B, :],
                        ones1_sb[:],
                        b2_sb[:, hq * 512 : (hq + 1) * 512],
                        start=True,
                        stop=False,
                        skip_group_check=True,
                        tile_position=(0, hq * B),
                    )

            def seg_a2x(s, t, xt):
                """Chain-free GEMM head: bias + xT K-tiles."""
                par = t % 2
                bias_quad(s, par)
                for k in range(4):
                    gemm_quad(s, par, k, xt[:, k, s * B : (s + 1) * B],
                              start=False, stop=False)

            def seg_b2(s, t, hNb):
                """hN transposes into hT (PE bf16) + one [128,128] copy."""
                if t >= T - 1:
                    return
                nc.tensor.transpose(tp_ps[s][:], hNb[:], i128_sb[:])
                nc.vector.tensor_copy(
                    hT_sb[s][:], tp_ps[s][:].rearrange("p (k n) -> p k n", n=B)
                )

            def seg_a1h(s, t):
                """scores + hT K-tiles, then mask/reduce/exp."""
                par = t % 2
                work = works[s]
                for j in range(4):
                    for q in range(4):
                        nc.tensor.matmul(
                            ms_ps[s][q * B : (q + 1) * B, 0:128],
                            hT_sb[s][:, j],
                            AhT_sb[:, j, s, q * 128 : (q + 1) * 128],
                            start=(j == 0),
                            stop=(j == 3),
                            skip_group_check=True,
                            tile_position=(0, q * B),
                        )
                for k in range(4):
                    gemm_quad(s, par, 4 + k, hT_sb[s][:, k], start=False, stop=False)

                Xm = work.tile([128, 128], F32, tag="Xm")
                nc.vector.tensor_mul(Xm[:], ms_ps[s][:, 0:128], mPN_sb[:])
                scSq = work.tile([128, 4], F32, tag="scSq")
                nc.vector.reduce_sum(
                    scSq[:],
                    Xm[:].rearrange("p (pl n) -> p pl n", n=B),
                    axis=mybir.AxisListType.X,
                )
                sgn = work.tile([128, 4], F32, tag="sgn")
                nc.scalar.activation(sgn[:], scSq[:], Sig, scale=-SCALE)
                rec = work.tile([128, 4], F32, tag="rec")
                nc.vector.reciprocal(rec[:], sgn[:])
                expS = work.tile([128, 4], F32, tag="expS")
                nc.vector.tensor_scalar(
                    expS[:], rec[:], -1.0, 0.0,
                    op0=mybir.AluOpType.add, op1=mybir.AluOpType.add,
                    accum_out=r128_sb[s][:],
                )
                return expS

            def seg_tail(s, t, expS):
                """den replication matmul + recip + normalize + transpose."""
                den = ms_ps[s][:, 288:289]
                nc.tensor.matmul(den, mQQ_sb[:], r128_sb[s][:], start=True, stop=True)
                nc.vector.reciprocal(rd128_sb[s][:], den)
                nc.vector.tensor_scalar_mul(wSn_sb[s][:, 0:4], expS[:], rd128_sb[s][:])
                nc.vector.transpose(wSTs_sb[s][:], wSn_sb[s][:])

            def seg_b1(s, t):
                """attention + attn GEMM K-tiles + gates/state update."""
                par = t % 2
                work = works[s]
                wBD_ps = ms_ps[s][:, 256:288]
                nc.tensor.matmul(
                    wBD_ps, d16x_sb[:], wSTs_sb[s][:], start=True, stop=True
                )
                wBDs = work.tile([128, B], BF16, tag="wBDs")
                nc.vector.tensor_mul(wBDs[:], wBD_ps, mBD_sb[:])

                for j in range(4):
                    at_j = ms_ps[s][:, 128 + j * B : 128 + (j + 1) * B]
                    for bb in range(NB):
                        nc.tensor.matmul(
                            at_j[:, bb * 8 : (bb + 1) * 8],
                            APT_sb[:, s, bb, j * 128 : (j + 1) * 128],
                            wBDs[:, bb * 8 : (bb + 1) * 8],
                            start=True,
                            stop=True,
                        )
                attnT = work.tile([128, 4, B], BF16, tag="attnT")
                nc.vector.tensor_copy(
                    attnT[:], ms_ps[s][:, 128:256].rearrange("p (k n) -> p k n", n=B)
                )

                for k in range(8, 12):
                    gemm_quad(s, par, k, attnT[:, k - 8], start=False, stop=(k == 11))

                gm = gm_ps[s][par]
                sg = work.tile([128, 384], BF16, tag="sg")
                nc.scalar.activation(sg[:], gm[:, 0:384], Sig)
                gg = work.tile([128, 128], BF16, tag="gg")
                nc.scalar.activation(gg[:], gm[:, 384:512], Tanh)
                fcp = work.tile([128, 128], F32, tag="fcp")
                nc.vector.tensor_mul(fcp[:], sg[:, 128:256], c_sb[s][:])
                igp = work.tile([128, 128], BF16, tag="igp")
                nc.vector.tensor_mul(igp[:], sg[:, 0:128], gg[:])
                nc.vector.tensor_add(c_sb[s][:], fcp[:], igp[:])
                tc_t = work.tile([128, 128], BF16, tag="tc")
                nc.scalar.activation(tc_t[:], c_sb[s][:], Tanh)
                hN = work.tile([128, 128], F32, tag="hN")
                nc.vector.tensor_mul(hN[:], sg[:, 256:384], tc_t[:])
                for hq in range(4):
                    nc.sync.dma_start(
                        out=out_d[s * B : (s + 1) * B, t, hq * 128 : (hq + 1) * 128],
                        in_=hN[hq * B : (hq + 1) * B, :],
                    )
                hNb = work.tile([128, 128], BF16, tag="hNb")
                nc.vector.tensor_copy(hNb[:], hN[:])
                return hNb

            # ---- slot pipeline over streams ----
            xts = {}
            xt0 = xin.tile([128, 4, NL], BF16, tag="xT")
            xts[0] = xt0
            nc.sync.dma_start(
                out=xt0[:], in_=xT_d[0].rearrange("(k p) n -> p k n", p=128)
            )

            exps, hnbs = {}, {}
            for u in range(2 * T + 2):
                sa = u % 2
                ta = u // 2
                sb = 1 - sa
                tb = (u - 1) // 2
                if ta < T:
                    if ta + 1 < T and (ta + 1) not in xts and sa == 1:
                        xtn = xin.tile([128, 4, NL], BF16, tag="xT")
                        xts[ta + 1] = xtn
                        nc.sync.dma_start(
                            out=xtn[:],
                            in_=xT_d[ta + 1].rearrange("(k p) n -> p k n", p=128),
                        )
                    seg_a2x(sa, ta, xts[ta])
                if u >= 1 and tb < T:
                    seg_tail(sb, tb, exps.pop((sb, tb)))
                    hnbs[(sb, tb)] = seg_b1(sb, tb)
                if ta - 1 >= 0 and (sa, ta - 1) in hnbs:
                    seg_b2(sa, ta - 1, hnbs.pop((sa, ta - 1)))
                if ta < T:
                    exps[(sa, ta)] = seg_a1h(sa, ta)

    if split_waits:
        _split_matmul_waits(nc)
    return nc


def _split_matmul_waits(nc):
    """Several TPB instruction encodings accept only one sync-wait command;
    hoist excess waits onto an inserted same-engine drain."""
    cnt = 0
    for f in nc.m.functions:
        for blk in f.blocks:
            new_insts = []
            for ins in blk.instructions:
                if (
                    ins.sync_info is not None
                    and ins.sync_info.on_wait
                    and len(ins.sync_info.on_wait) > 1
                ):
                    waits = list(ins.sync_info.on_wait)
                    for w in waits[:-1]:
                        cnt += 1
                        d = mybir.InstDrain(
                            name=f"I-mmw{cnt}", ins=[], outs=[],
                            engine=ins.engine,
                        )
                        d.sync_info = mybir.SyncInfo(on_wait=[w], on_update=[])
                        new_insts.append(d)
                    ins.sync_info = mybir.SyncInfo(
                        on_wait=[waits[-1]], on_update=list(ins.sync_info.on_update or [])
                    )
                new_insts.append(ins)
            blk.instructions = new_insts


def _prep_core_inputs(x_i, A_i, Wx, Wh, Wattn, b):
    """Host-side layout prep for one core's shard (x_i: (64,T,D), A_i: (64,H,4,4))."""
    nl = x_i.shape[0]
    A_flat = A_i.reshape(nl, H, P16)
    h0 = A_flat.mean(axis=2).astype(np.float32)  # (64, H)

    xT = np.ascontiguousarray(x_i.transpose(1, 2, 0)).astype(np.float32)  # (T, D, 64)
    # AhT[h, s, p*32+n] = A_flat[32s+n, h, p]
    AhT = np.ascontiguousarray(
        A_flat.transpose(1, 2, 0).reshape(H, P16, 2, B).transpose(0, 2, 1, 3)
        .reshape(H, 2, P16 * B)
    ).astype(np.float32)
    # APT[p*8+r, s, b, h] = A_flat[32s + 8b + r, h, p]
    APT = np.ascontiguousarray(
        A_flat.reshape(2, NB, 8, H, P16).transpose(4, 2, 0, 1, 3)
        .reshape(128, 2, NB, H)
    ).astype(np.float32)
    # W columns permuted: W2[:, hq*512 + g*128 + h1] = W[:, g*512 + hq*128 + h1]
    W = np.concatenate([Wx, Wh, Wattn], axis=0).astype(np.float32)  # (1536, E)
    W2 = np.ascontiguousarray(
        W.reshape(3 * H, 4, 4, 128).transpose(0, 2, 1, 3).reshape(3 * H, E)
    )
    b2 = np.ascontiguousarray(
        b.reshape(4, 4, 128).transpose(1, 0, 2).reshape(1, E)
    ).astype(np.float32)
    # c0[s, hq*32+n, h1] = h0[s*32+n, hq*128+h1]
    c0 = np.ascontiguousarray(
        h0.reshape(2, B, 4, 128).transpose(0, 2, 1, 3).reshape(2, 128, 128)
    )
    i128 = np.eye(128, dtype=np.float32)
    # mPN[q*32+m, pl*32+n] = (n == m)
    mPN = np.tile(np.tile(np.eye(B, dtype=np.float32), (1, 4)), (4, 1))  # (128, 128)
    mBD = np.tile(np.tile(np.eye(8, dtype=np.float32), (1, NB)), (P16, 1))  # (128,32)
    # d16x[q*32+pl, p*8+r] = (p == q*4+pl), pl<4
    d16x = np.zeros((128, 128), dtype=np.float32)
    for p in range(P16):
        q, pl = p // 4, p % 4
        d16x[q * 32 + pl, p * 8 : (p + 1) * 8] = 1.0
    mQQ = np.tile(np.eye(B, dtype=np.float32), (4, 4))  # (128, 128)
    ones1 = np.ones((1, B), dtype=np.float32)
    bf16 = ml_dtypes.bfloat16
    return {
        "xT": xT.astype(bf16),
        "AhT": AhT.astype(bf16),
        "APT": APT.astype(bf16),
        "W2": W2.astype(bf16),
        "b2": b2.astype(bf16),
        "c0": c0,
        "h0T": np.ascontiguousarray(h0.T).astype(bf16),
        "i128": i128.astype(bf16),
        "mPN": mPN,
        "mBD": mBD.astype(bf16),
        "d16x": d16x.astype(bf16),
        "mQQ": mQQ,
        "ones1": ones1.astype(bf16),
    }


_NC_CACHE = {}


def kernel(x, A, Wx, Wh, Wattn, b, _trace=False):
    x = np.asarray(x, dtype=np.float32)
    A = np.asarray(A, dtype=np.float32)
    Wx = np.asarray(Wx, dtype=np.float32)
    Wh = np.asarray(Wh, dtype=np.float32)
    Wattn = np.asarray(Wattn, dtype=np.float32)
    b = np.asarray(b, dtype=np.float32)

    if "nc" not in _NC_CACHE:
        _NC_CACHE["nc"] = build_nc()
    nc = _NC_CACHE["nc"]

    in_maps = []
    for i in range(NCORES):
        sl = slice(i * NL, (i + 1) * NL)
        in_maps.append(_prep_core_inputs(x[sl], A[sl], Wx, Wh, Wattn, b))

    res = run_bass_kernel_spmd(
        nc, in_maps, core_ids=list(range(NCORES)), trace=_trace
    )
    outs = [res.results[i]["out"] for i in range(NCORES)]
    full = np.concatenate(outs, axis=0)  # (N, T, H)
    if _trace:
        kernel.last_exec_time_ns = res.exec_time_ns
        kernel.last_profile = res.profile_json
    return full


kernel.last_exec_time_ns = None
kernel.last_profile = None
